# revision 19
# baseline (speedup 1.0000x reference)
"""CoarseMatching (LoFTR-style dual-softmax matching) on 8 Trainium2 cores.

Sharding: each core owns 600 rows (L dim) of both pairs (N=2).  Per pair:
sim = (f0 @ f1^T) / (C^0.5 * C^0.5 * TEMP) computed with fp32r matmuls,
P = exp(sim) kept resident, row sums via ACT accum, column sums via
ones-matmul on PE + one 8-core AllReduce, conf = (P*rsqrt(rsum))^2 * cinv
written back to HBM.  Host assembles shards and applies the (empty in
practice) threshold/border/mutual-NN masking.
"""

import sys
import numpy as np

sys.path.insert(0, "/opt/trn_rl_repo")

import concourse.bacc as bacc
import concourse.bass as bass
import concourse.tile as tile
from concourse import mybir
from concourse.bass_utils import run_bass_kernel_spmd
from concourse import hw_specs as _hw_specs

# Pin every ACT function to the one table set that contains all of
# {exp, ln, square, copy, identity} so the kernel does a single
# ACT_TABLE_LOAD instead of thrashing between sets (2.7us per switch).
_orig_get_activation_tables = _hw_specs.get_activation_tables

def _pinned_activation_tables(module_arch):
    tables = _orig_get_activation_tables(module_arch)
    keep = "natural_log_exp_and_others"
    return {
        name: (funcs if name == keep else set())
        for name, funcs in tables.items()
    }

bacc.get_activation_tables = _pinned_activation_tables

N_CORES = 8
N, L, S, C = 2, 4800, 4800, 256
H0, W0, H1, W1 = 60, 80, 60, 80
THR = 0.2
BORDER_RM = 2
TEMP = 0.1
SCALE = 1.0 / (C * TEMP)  # folded into f0T: (1/sqrt(C))^2 / TEMP
K0 = 8.67  # ~ln(S * E[exp(sim)]): centers ln(csum) so fp32r rounding of the
           # rank-1 operand loses no precision for typical inputs

SHARD = L // N_CORES          # 600 rows per core per pair
LP = 120                      # rows per l-tile (partition dim)
NLT = SHARD // LP             # 5 l-tiles
NBW = 480                     # columns per matmul block
NNB = S // NBW                # 10 column blocks
EBW = 960                     # columns per exp/psum superblock (2 banks)
NEB = S // EBW                # 5 exp blocks
TBW = 960                     # columns per conf TT/DMA block
NTB = S // TBW
KC = 128                      # contraction chunk (partitions)
NKC = C // KC                 # 2 chunks

F32 = mybir.dt.float32
F32R = mybir.dt.float32r


def _row_blocks(total):
    blocks = []
    start = 0
    while start < total:
        sz = min(128, total - start)
        blocks.append((start, sz))
        start += sz
    return blocks


def build_nc():
    nc = bacc.Bacc("TRN2", target_bir_lowering=False, num_devices=N_CORES)

    f0t = nc.declare_dram_parameter("f0t", [N, C, SHARD], F32, isOutput=False)
    f1t = nc.declare_dram_parameter("f1t", [N, C, S], F32, isOutput=False)
    ones_in = nc.declare_dram_parameter("ones_in", [128, 128], F32, isOutput=False)
    conf_o = nc.declare_dram_parameter("conf_o", [N, SHARD, S], F32, isOutput=True)

    with tile.TileContext(nc) as tc:
        with (
            tc.tile_pool(name="single", bufs=1) as single,
            tc.tile_pool(name="f1T", bufs=2) as f1T_pool,
            tc.tile_pool(name="f0T", bufs=4) as f0T_pool,
            tc.tile_pool(name="Pp", bufs=14) as P_pool,
            tc.tile_pool(name="sin", bufs=3) as sin_pool,
            tc.tile_pool(name="stats", bufs=12) as stats_pool,
            tc.tile_pool(name="tiny", bufs=24) as tiny_pool,
            tc.tile_pool(name="cs", bufs=2) as cs_pool,
            tc.tile_pool(name="f1rB", bufs=4) as f1rB_pool,
            tc.tile_pool(name="lrow", bufs=1) as lrow_pool,
            tc.tile_pool(name="stage", bufs=3) as stage_pool,
            tc.tile_pool(name="ps", bufs=3, space="PSUM") as ps_pool,
            tc.tile_pool(name="pc", bufs=2, space="PSUM") as pc_pool,
            tc.tile_pool(name="dram", bufs=1, space="DRAM") as dram_pool,
        ):
            ones_f = single.tile([128, 128], F32)
            nc.sync.dma_start(out=ones_f, in_=ones_in[:, :])
            ones_r = single.tile([128, 128], F32R)
            nc.vector.tensor_copy(ones_r, ones_f)

            cc_in = []
            cc_out = []
            stag = []
            for p in range(N):
                t_in = dram_pool.tile([S], F32, name=f"cc_in{p}", tag=f"cc_in{p}")
                t_out = dram_pool.tile([S], F32, name=f"cc_out{p}", addr_space="Shared", tag=f"cc_out{p}")
                t_st = dram_pool.tile([S], F32, name=f"stag{p}", tag=f"stag{p}")
                cc_in.append(t_in)
                cc_out.append(t_out)
                stag.append(t_st)

            def load_transposed(p):
                """Load host-pre-transposed features via HWDGE into f32
                staging chunks, cast to fp32r on DVE (2x mode)."""
                f0T = []
                f1T = []
                for k in range(NKC):
                    t0 = f0T_pool.tile([128, SHARD], F32R, name=f"f0T{p}{k}", tag="f0T")
                    st0 = sin_pool.tile([128, EBW], F32, name="st0", tag="sin")
                    nc.sync.dma_start(
                        out=st0[:, :SHARD], in_=f0t[p, k * KC : (k + 1) * KC, :]
                    )
                    nc.vector.tensor_copy(t0, st0[:, :SHARD])
                    f0T.append(t0)
                    t1 = f1T_pool.tile([128, S], F32R, name=f"f1T{p}{k}", tag="f1T")
                    f1T.append(t1)
                for b in range(NEB):
                    b0 = b * EBW
                    for k in range(NKC):
                        st1 = sin_pool.tile([128, EBW], F32, name="st1", tag="sin")
                        nc.sync.dma_start(
                            out=st1, in_=f1t[p, k * KC : (k + 1) * KC, b0 : b0 + EBW]
                        )
                        nc.vector.tensor_copy(f1T[k][:, b0 : b0 + EBW], st1)
                return f0T, f1T

            def phase_a(p, f0T, f1T):
                """sim matmuls -> exp into resident P blocks + row sums;
                column-sum matmuls fire as soon as each column block is done
                so the AllReduce can start right at phase-A end."""
                P_blocks = {}
                rsp_tiles = []
                for lt in range(NLT):
                    rsp = stats_pool.tile([LP, 16], F32, name=f"rsp{lt}", tag="rsp")
                    rsp_tiles.append(rsp)
                for eb in range(NEB):
                    e0 = eb * EBW
                    for lt in range(NLT):
                        lc = lt * LP
                        P_blk = P_pool.tile([LP, EBW], F32R, name=f"P{lt}_{eb}", tag="P")
                        P_blocks[(lt, eb)] = P_blk
                        # two bank-aligned 512-wide regions; 480 cols used each
                        pst = ps_pool.tile([LP, 2, 512], F32, name="pst")
                        for k in range(NKC):
                            for h in range(EBW // NBW):
                                nc.tensor.matmul(
                                    pst[:, h, 0:NBW],
                                    f0T[k][:, lc : lc + LP],
                                    f1T[k][:, e0 + h * NBW : e0 + (h + 1) * NBW],
                                    start=(k == 0),
                                    stop=(k == NKC - 1),
                                )
                        nc.scalar.activation(
                            P_blk.rearrange("p (a b) -> p a b", a=2),
                            pst[:, :, 0:NBW],
                            mybir.ActivationFunctionType.Exp,
                            accum_out=rsp_tiles[lt][:, eb : eb + 1],
                        )
                    # column sums for this eb column (2 nb blocks of 480)
                    for half in range(2):
                        nb = 2 * eb + half
                        off = half * NBW
                        pct = pc_pool.tile([128, NBW], F32, name="pct")
                        for lt in range(NLT):
                            nc.tensor.matmul(
                                pct,
                                ones_r[:LP, :],
                                P_blocks[(lt, eb)][:, off : off + NBW],
                                start=(lt == 0),
                                stop=(lt == NLT - 1),
                            )
                        csb = cs_pool.tile([1, NBW], F32, name="csb")
                        nc.scalar.copy(csb, pct[0:1, :])
                        nc.gpsimd.dma_start(
                            out=cc_in[p][nb * NBW : (nb + 1) * NBW], in_=csb
                        )
                bias_tiles = []
                for lt in range(NLT):
                    # bias = -ln(rsum) - K0  (per-partition, for the conf exp)
                    rs1 = tiny_pool.tile([LP, 1], F32, name=f"rs1_{lt}", tag="rs1")
                    nc.vector.tensor_reduce(
                        rs1, rsp_tiles[lt][:, 0:NEB], axis=mybir.AxisListType.X,
                        op=mybir.AluOpType.add,
                    )
                    ln1 = tiny_pool.tile([LP, 1], F32, name=f"ln1_{lt}", tag="ln1")
                    nc.scalar.activation(ln1, rs1, mybir.ActivationFunctionType.Ln)
                    blt = tiny_pool.tile([LP, 1], F32, name=f"blt{lt}", tag="blt")
                    nc.vector.tensor_scalar(
                        blt, ln1, -1.0, -K0,
                        op0=mybir.AluOpType.mult, op1=mybir.AluOpType.add,
                    )
                    bias_tiles.append(blt)
                return P_blocks, bias_tiles

            def csum_and_allreduce(p, P_blocks):
                """8-core AllReduce of the column sums."""
                nc.gpsimd.collective_compute(
                    "AllReduce",
                    mybir.AluOpType.add,
                    replica_groups=[list(range(N_CORES))],
                    ins=[cc_in[p][:].opt()],
                    outs=[cc_out[p][:].opt()],
                )

            def lncs_row(p):
                """r[s] = -0.5*(ln(csum[s]) - K0) as an fp32r row for the
                rank-1 PSUM correction."""
                csr = cs_pool.tile([96, 50], F32, name="csr")
                nc.gpsimd.dma_start(
                    out=csr, in_=cc_out[p][:].rearrange("(a b) -> a b", a=96)
                )
                lnl = cs_pool.tile([96, 50], F32, name="lnl")
                nc.scalar.activation(lnl, csr, mybir.ActivationFunctionType.Ln)
                adj = cs_pool.tile([96, 50], F32, name="adj")
                nc.vector.tensor_scalar(
                    adj, lnl, -K0, -0.5,
                    op0=mybir.AluOpType.add, op1=mybir.AluOpType.mult,
                )
                nc.gpsimd.dma_start(
                    out=stag[p][:].rearrange("(a b) -> a b", a=96), in_=adj
                )
                lrow_f = lrow_pool.tile([1, S], F32, name="lrow_f", tag="lrow_f")
                nc.gpsimd.dma_start(
                    out=lrow_f, in_=stag[p][:].rearrange("(a b) -> a b", a=1)
                )
                lrow_r = lrow_pool.tile([1, S], F32R, name="lrow_r", tag="lrow_r")
                nc.vector.tensor_copy(lrow_r, lrow_f)
                return lrow_r

            def phase_b(p, f0T, bias_tiles, lrow_r):
                """conf = exp(2*sim - ln rsum - ln csum), straight from PSUM:
                recompute sim, add the rank-1 csum correction in PSUM, and
                let the activation apply scale/bias.  No P reads, no DVE."""
                for eb in range(NEB):
                    e0 = eb * EBW
                    fB = []
                    for k in range(NKC):
                        st1 = sin_pool.tile([128, EBW], F32, name="st1", tag="sin")
                        nc.sync.dma_start(
                            out=st1, in_=f1t[p, k * KC : (k + 1) * KC, e0 : e0 + EBW]
                        )
                        t = f1rB_pool.tile([128, EBW], F32R, name="fB", tag="f1rB")
                        nc.vector.tensor_copy(t, st1)
                        fB.append(t)
                    for lt in range(NLT):
                        lc = lt * LP
                        pst = ps_pool.tile([LP, 2, 512], F32, name="pst")
                        for k in range(NKC):
                            for h in range(EBW // NBW):
                                nc.tensor.matmul(
                                    pst[:, h, 0:NBW],
                                    f0T[k][:, lc : lc + LP],
                                    fB[k][:, h * NBW : (h + 1) * NBW],
                                    start=(k == 0),
                                    stop=False,
                                )
                        for h in range(EBW // NBW):
                            nc.tensor.matmul(
                                pst[:, h, 0:NBW],
                                ones_r[0:1, :LP],
                                lrow_r[0:1, e0 + h * NBW : e0 + (h + 1) * NBW],
                                start=False,
                                stop=True,
                            )
                        st = stage_pool.tile([LP, EBW], F32, name="st")
                        nc.scalar.activation(
                            st.rearrange("p (a b) -> p a b", a=2),
                            pst[:, :, 0:NBW],
                            mybir.ActivationFunctionType.Exp,
                            scale=2.0,
                            bias=bias_tiles[lt],
                        )
                        nc.sync.dma_start(
                            out=conf_o[p, lt * LP : (lt + 1) * LP, e0 : e0 + EBW],
                            in_=st,
                        )

            # Interleave the two pairs so pair-1 load/transpose/compute fills
            # the pair-0 AllReduce + phase-B window (PE stays warm).
            f0T0, f1T0 = load_transposed(0)
            P0, b0 = phase_a(0, f0T0, f1T0)
            csum_and_allreduce(0, P0)
            f0T1, f1T1 = load_transposed(1)
            P1, b1 = phase_a(1, f0T1, f1T1)
            r0 = lncs_row(0)
            phase_b(0, f0T0, b0, r0)
            csum_and_allreduce(1, P1)
            r1 = lncs_row(1)
            phase_b(1, f0T1, b1, r1)

    nc.compile()
    return nc


_CACHED = {}


def _get_nc():
    if "nc" not in _CACHED:
        _CACHED["nc"] = build_nc()
    return _CACHED["nc"]


def run_device(feat_c0, feat_c1, trace=False, tmpdir=None):
    """Runs the SPMD kernel; returns (conf [N,L,S], results obj)."""
    nc = _get_nc()
    ones = np.ones((128, 128), dtype=np.float32)
    # host-side layout prep: scale one side by the softmax/temp factor and
    # put the contraction dim on partitions for both operands
    f0t_full = np.ascontiguousarray(
        (feat_c0 * np.float32(SCALE)).transpose(0, 2, 1)
    )  # [N, C, L]
    f1t = np.ascontiguousarray(feat_c1.transpose(0, 2, 1))  # [N, C, S]
    in_maps = []
    for c in range(N_CORES):
        rs = c * SHARD
        in_maps.append(
            {
                "f0t": np.ascontiguousarray(f0t_full[:, :, rs : rs + SHARD]),
                "f1t": f1t,
                "ones_in": ones,
            }
        )
    res = run_bass_kernel_spmd(
        nc, in_maps, list(range(N_CORES)), trace=trace, tmpdir=tmpdir
    )
    conf = np.empty((N, L, S), dtype=np.float32)
    for c in range(N_CORES):
        rs = c * SHARD
        conf[:, rs : rs + SHARD, :] = res.results[c]["conf_o"]
    return conf, res


def _interior(n, idx):
    return (idx >= BORDER_RM) & (idx < n - BORDER_RM)


def kernel(feat_c0, feat_c1, h0c, w0c, h1c, w1c):
    feat_c0 = np.asarray(feat_c0, dtype=np.float32)
    feat_c1 = np.asarray(feat_c1, dtype=np.float32)
    h0c, w0c, h1c, w1c = int(h0c), int(w0c), int(h1c), int(w1c)
    assert feat_c0.shape == (N, L, C) and feat_c1.shape == (N, S, C)
    assert (h0c * w0c, h1c * w1c) == (L, S)

    conf, _ = run_device(feat_c0, feat_c1)
    rowmax = conf.max(axis=2)

    # ---- host finalize: threshold + border + mutual-NN (tiny) ----
    mask_v = np.zeros((N, L), dtype=bool)
    j_ids = np.zeros((N, L), dtype=np.int32)
    mconf = np.zeros((N, L), dtype=np.float32)

    cand_n, cand_l = np.nonzero(rowmax > THR)
    for n_i, l_i in zip(cand_n, cand_l):
        # row border (interior of the h0c x w0c grid)
        if not (_interior(h0c, l_i // w0c) and _interior(w0c, l_i % w0c)):
            continue
        row = conf[n_i, l_i]
        j = int(np.argmax(row))
        v = row[j]
        if not (v > THR):
            continue
        # column border (interior of the h1c x w1c grid)
        if not (_interior(h1c, j // w1c) and _interior(w1c, j % w1c)):
            continue
        # mutual nearest neighbor: also the max of its column
        if conf[n_i, :, j].max() != v:
            continue
        mask_v[n_i, l_i] = True
        j_ids[n_i, l_i] = j
        mconf[n_i, l_i] = v

    mkpts1_c = np.stack([j_ids % w1c, j_ids // w1c], axis=-1).astype(np.int32)
    return conf, mask_v, j_ids, mkpts1_c, mconf


# revision 20
# speedup vs baseline: 1.2748x; 1.2748x over previous
"""CoarseMatching (LoFTR-style dual-softmax matching) on 8 Trainium2 cores.

Sharding: each core owns 600 rows (L dim) of both pairs (N=2).  Per pair:
sim = (f0 @ f1^T) / (C^0.5 * C^0.5 * TEMP) computed with fp32r matmuls,
P = exp(sim) kept resident, row sums via ACT accum, column sums via
ones-matmul on PE + one 8-core AllReduce, conf = (P*rsqrt(rsum))^2 * cinv
written back to HBM.  Host assembles shards and applies the (empty in
practice) threshold/border/mutual-NN masking.
"""

import sys
import numpy as np

sys.path.insert(0, "/opt/trn_rl_repo")

import concourse.bacc as bacc
import concourse.bass as bass
import concourse.tile as tile
from concourse import mybir
from concourse.bass_utils import run_bass_kernel_spmd
from concourse import hw_specs as _hw_specs

# Pin every ACT function to the one table set that contains all of
# {exp, ln, square, copy, identity} so the kernel does a single
# ACT_TABLE_LOAD instead of thrashing between sets (2.7us per switch).
_orig_get_activation_tables = _hw_specs.get_activation_tables

def _pinned_activation_tables(module_arch):
    tables = _orig_get_activation_tables(module_arch)
    keep = "natural_log_exp_and_others"
    return {
        name: (funcs if name == keep else set())
        for name, funcs in tables.items()
    }

bacc.get_activation_tables = _pinned_activation_tables

N_CORES = 8
N, L, S, C = 2, 4800, 4800, 256
H0, W0, H1, W1 = 60, 80, 60, 80
THR = 0.2
BORDER_RM = 2
TEMP = 0.1
SCALE = 1.0 / (C * TEMP)  # folded into f0T: (1/sqrt(C))^2 / TEMP
K0 = 8.67  # ~ln(S * E[exp(sim)]): centers ln(csum) so fp32r rounding of the
           # rank-1 operand loses no precision for typical inputs

SHARD = L // N_CORES          # 600 rows per core per pair
LP = 120                      # rows per l-tile (partition dim)
NLT = SHARD // LP             # 5 l-tiles
NBW = 480                     # columns per matmul block
NNB = S // NBW                # 10 column blocks
EBW = 960                     # columns per exp/psum superblock (2 banks)
NEB = S // EBW                # 5 exp blocks
TBW = 960                     # columns per conf TT/DMA block
NTB = S // TBW
KC = 128                      # contraction chunk (partitions)
NKC = C // KC                 # 2 chunks

F32 = mybir.dt.float32
F32R = mybir.dt.float32r


def _row_blocks(total):
    blocks = []
    start = 0
    while start < total:
        sz = min(128, total - start)
        blocks.append((start, sz))
        start += sz
    return blocks


def build_nc():
    nc = bacc.Bacc("TRN2", target_bir_lowering=False, num_devices=N_CORES)

    f0t = nc.declare_dram_parameter("f0t", [N, C, SHARD], F32, isOutput=False)
    f1t = nc.declare_dram_parameter("f1t", [N, C, S], F32, isOutput=False)
    ones_in = nc.declare_dram_parameter("ones_in", [128, 128], F32, isOutput=False)
    conf_o = nc.declare_dram_parameter("conf_o", [N, SHARD, S], F32, isOutput=True)

    with tile.TileContext(nc) as tc:
        with (
            tc.tile_pool(name="single", bufs=1) as single,
            tc.tile_pool(name="f1T", bufs=2) as f1T_pool,
            tc.tile_pool(name="f0T", bufs=2) as f0T_pool,
            tc.tile_pool(name="Pp", bufs=26) as P_pool,
            tc.tile_pool(name="sin", bufs=3) as sin_pool,
            tc.tile_pool(name="stats", bufs=12) as stats_pool,
            tc.tile_pool(name="tiny", bufs=24) as tiny_pool,
            tc.tile_pool(name="cs", bufs=2) as cs_pool,
            tc.tile_pool(name="cb", bufs=1) as cb_pool,
            tc.tile_pool(name="stage", bufs=3) as stage_pool,
            tc.tile_pool(name="ps", bufs=3, space="PSUM") as ps_pool,
            tc.tile_pool(name="pc", bufs=2, space="PSUM") as pc_pool,
            tc.tile_pool(name="dram", bufs=1, space="DRAM") as dram_pool,
        ):
            ones_f = single.tile([128, 128], F32)
            nc.sync.dma_start(out=ones_f, in_=ones_in[:, :])
            ones_r = single.tile([128, 128], F32R)
            nc.vector.tensor_copy(ones_r, ones_f)

            cc_in = []
            cc_out = []
            stag = []
            for p in range(N):
                t_in = dram_pool.tile([S], F32, name=f"cc_in{p}", tag=f"cc_in{p}")
                t_out = dram_pool.tile([S], F32, name=f"cc_out{p}", addr_space="Shared", tag=f"cc_out{p}")
                t_st = dram_pool.tile([S], F32, name=f"stag{p}", tag=f"stag{p}")
                cc_in.append(t_in)
                cc_out.append(t_out)
                stag.append(t_st)

            def load_transposed(p):
                """Load host-pre-transposed features via HWDGE into f32
                staging chunks, cast to fp32r on DVE (2x mode)."""
                f0T = []
                f1T = []
                for k in range(NKC):
                    t0 = f0T_pool.tile([128, SHARD], F32R, name=f"f0T{p}{k}", tag="f0T")
                    st0 = sin_pool.tile([128, EBW], F32, name="st0", tag="sin")
                    nc.sync.dma_start(
                        out=st0[:, :SHARD], in_=f0t[p, k * KC : (k + 1) * KC, :]
                    )
                    nc.vector.tensor_copy(t0, st0[:, :SHARD])
                    f0T.append(t0)
                    t1 = f1T_pool.tile([128, S], F32R, name=f"f1T{p}{k}", tag="f1T")
                    f1T.append(t1)
                for b in range(NEB):
                    b0 = b * EBW
                    for k in range(NKC):
                        st1 = sin_pool.tile([128, EBW], F32, name="st1", tag="sin")
                        nc.sync.dma_start(
                            out=st1, in_=f1t[p, k * KC : (k + 1) * KC, b0 : b0 + EBW]
                        )
                        nc.vector.tensor_copy(f1T[k][:, b0 : b0 + EBW], st1)
                return f0T, f1T

            def phase_a(p, f0T, f1T):
                """sim matmuls -> exp into resident P blocks + row sums;
                column-sum matmuls fire as soon as each column block is done
                so the AllReduce can start right at phase-A end."""
                P_blocks = {}
                rsp_tiles = []
                for lt in range(NLT):
                    rsp = stats_pool.tile([LP, 16], F32, name=f"rsp{lt}", tag="rsp")
                    rsp_tiles.append(rsp)
                for eb in range(NEB):
                    e0 = eb * EBW
                    for lt in range(NLT):
                        lc = lt * LP
                        P_blk = P_pool.tile([LP, EBW], F32R, name=f"P{lt}_{eb}", tag="P")
                        P_blocks[(lt, eb)] = P_blk
                        # two bank-aligned 512-wide regions; 480 cols used each
                        pst = ps_pool.tile([LP, 2, 512], F32, name="pst")
                        for k in range(NKC):
                            for h in range(EBW // NBW):
                                nc.tensor.matmul(
                                    pst[:, h, 0:NBW],
                                    f0T[k][:, lc : lc + LP],
                                    f1T[k][:, e0 + h * NBW : e0 + (h + 1) * NBW],
                                    start=(k == 0),
                                    stop=(k == NKC - 1),
                                )
                        nc.scalar.activation(
                            P_blk.rearrange("p (a b) -> p a b", a=2),
                            pst[:, :, 0:NBW],
                            mybir.ActivationFunctionType.Exp,
                            accum_out=rsp_tiles[lt][:, eb : eb + 1],
                        )
                    # column sums for this eb column (2 nb blocks of 480)
                    for half in range(2):
                        nb = 2 * eb + half
                        off = half * NBW
                        pct = pc_pool.tile([128, NBW], F32, name="pct")
                        for lt in range(NLT):
                            nc.tensor.matmul(
                                pct,
                                ones_r[:LP, :],
                                P_blocks[(lt, eb)][:, off : off + NBW],
                                start=(lt == 0),
                                stop=(lt == NLT - 1),
                            )
                        csb = cs_pool.tile([1, NBW], F32, name="csb")
                        nc.scalar.copy(csb, pct[0:1, :])
                        nc.gpsimd.dma_start(
                            out=cc_in[p][nb * NBW : (nb + 1) * NBW], in_=csb
                        )
                srinv_tiles = []
                for lt in range(NLT):
                    # rsum -> srinv = exp(-0.5 * ln(rsum))
                    rs1 = tiny_pool.tile([LP, 1], F32, name=f"rs1_{lt}", tag="rs1")
                    nc.vector.tensor_reduce(
                        rs1, rsp_tiles[lt][:, 0:NEB], axis=mybir.AxisListType.X,
                        op=mybir.AluOpType.add,
                    )
                    ln1 = tiny_pool.tile([LP, 1], F32, name=f"ln1_{lt}", tag="ln1")
                    nc.scalar.activation(ln1, rs1, mybir.ActivationFunctionType.Ln)
                    srinv = tiny_pool.tile([LP, 1], F32, name=f"srinv{lt}", tag="srinv")
                    nc.scalar.activation(
                        srinv, ln1, mybir.ActivationFunctionType.Exp, scale=-0.5
                    )
                    srinv_tiles.append(srinv)
                return P_blocks, srinv_tiles

            def csum_and_allreduce(p, P_blocks):
                """8-core AllReduce of the column sums."""
                nc.gpsimd.collective_compute(
                    "AllReduce",
                    mybir.AluOpType.add,
                    replica_groups=[list(range(N_CORES))],
                    ins=[cc_in[p][:].opt()],
                    outs=[cc_out[p][:].opt()],
                )

            def cinv_broadcast(p):
                """cinv = 1/csum on 96 lanes, then DMA-broadcast to 128 parts."""
                csr = cs_pool.tile([96, 50], F32, name="csr")
                nc.gpsimd.dma_start(
                    out=csr, in_=cc_out[p][:].rearrange("(a b) -> a b", a=96)
                )
                cis = cs_pool.tile([96, 50], F32, name="cis")
                nc.vector.reciprocal(cis, csr)
                nc.gpsimd.dma_start(
                    out=stag[p][:].rearrange("(a b) -> a b", a=96), in_=cis
                )
                cb_t = cb_pool.tile([128, S], F32, name="cb_t")
                stag_ap = stag[p][:]
                stag_bcast = bass.AP(
                    tensor=stag_ap.tensor,
                    offset=stag_ap.offset,
                    ap=[[0, 128]] + list(stag_ap.ap),
                )
                nc.gpsimd.dma_start(out=cb_t, in_=stag_bcast)
                return cb_t

            def phase_b(p, P_blocks, srinv_tiles, cb_t):
                """conf = (P * srinv)^2 * cinv -> HBM."""
                for lt in range(NLT):
                    lr = lt * LP
                    for eb in range(NEB):
                        e0 = eb * EBW
                        P_blk = P_blocks[(lt, eb)]
                        nc.scalar.activation(
                            P_blk,
                            P_blk.bitcast(F32),
                            mybir.ActivationFunctionType.Square,
                            scale=srinv_tiles[lt],
                        )
                        st = stage_pool.tile([LP, EBW], F32, name="st")
                        nc.vector.tensor_mul(
                            st,
                            P_blk.bitcast(F32),
                            cb_t[:LP, e0 : e0 + EBW],
                        )
                        nc.sync.dma_start(
                            out=conf_o[p, lr : lr + LP, e0 : e0 + EBW], in_=st
                        )

            # Interleave the two pairs so pair-1 load/transpose/compute fills
            # the pair-0 AllReduce + phase-B window (PE stays warm).
            f0T0, f1T0 = load_transposed(0)
            P0, sr0 = phase_a(0, f0T0, f1T0)
            csum_and_allreduce(0, P0)
            f0T1, f1T1 = load_transposed(1)
            cb0_t = cinv_broadcast(0)
            phase_b(0, P0, sr0, cb0_t)
            P1, sr1 = phase_a(1, f0T1, f1T1)
            csum_and_allreduce(1, P1)
            cb1_t = cinv_broadcast(1)
            phase_b(1, P1, sr1, cb1_t)

    nc.compile()
    return nc


_CACHED = {}


def _get_nc():
    if "nc" not in _CACHED:
        _CACHED["nc"] = build_nc()
    return _CACHED["nc"]


def run_device(feat_c0, feat_c1, trace=False, tmpdir=None):
    """Runs the SPMD kernel; returns (conf [N,L,S], results obj)."""
    nc = _get_nc()
    ones = np.ones((128, 128), dtype=np.float32)
    # host-side layout prep: scale one side by the softmax/temp factor and
    # put the contraction dim on partitions for both operands
    f0t_full = np.ascontiguousarray(
        (feat_c0 * np.float32(SCALE)).transpose(0, 2, 1)
    )  # [N, C, L]
    f1t = np.ascontiguousarray(feat_c1.transpose(0, 2, 1))  # [N, C, S]
    in_maps = []
    for c in range(N_CORES):
        rs = c * SHARD
        in_maps.append(
            {
                "f0t": np.ascontiguousarray(f0t_full[:, :, rs : rs + SHARD]),
                "f1t": f1t,
                "ones_in": ones,
            }
        )
    res = run_bass_kernel_spmd(
        nc, in_maps, list(range(N_CORES)), trace=trace, tmpdir=tmpdir
    )
    conf = np.empty((N, L, S), dtype=np.float32)
    for c in range(N_CORES):
        rs = c * SHARD
        conf[:, rs : rs + SHARD, :] = res.results[c]["conf_o"]
    return conf, res


def _interior(n, idx):
    return (idx >= BORDER_RM) & (idx < n - BORDER_RM)


def kernel(feat_c0, feat_c1, h0c, w0c, h1c, w1c):
    feat_c0 = np.asarray(feat_c0, dtype=np.float32)
    feat_c1 = np.asarray(feat_c1, dtype=np.float32)
    h0c, w0c, h1c, w1c = int(h0c), int(w0c), int(h1c), int(w1c)
    assert feat_c0.shape == (N, L, C) and feat_c1.shape == (N, S, C)
    assert (h0c * w0c, h1c * w1c) == (L, S)

    conf, _ = run_device(feat_c0, feat_c1)
    rowmax = conf.max(axis=2)

    # ---- host finalize: threshold + border + mutual-NN (tiny) ----
    mask_v = np.zeros((N, L), dtype=bool)
    j_ids = np.zeros((N, L), dtype=np.int32)
    mconf = np.zeros((N, L), dtype=np.float32)

    cand_n, cand_l = np.nonzero(rowmax > THR)
    for n_i, l_i in zip(cand_n, cand_l):
        # row border (interior of the h0c x w0c grid)
        if not (_interior(h0c, l_i // w0c) and _interior(w0c, l_i % w0c)):
            continue
        row = conf[n_i, l_i]
        j = int(np.argmax(row))
        v = row[j]
        if not (v > THR):
            continue
        # column border (interior of the h1c x w1c grid)
        if not (_interior(h1c, j // w1c) and _interior(w1c, j % w1c)):
            continue
        # mutual nearest neighbor: also the max of its column
        if conf[n_i, :, j].max() != v:
            continue
        mask_v[n_i, l_i] = True
        j_ids[n_i, l_i] = j
        mconf[n_i, l_i] = v

    mkpts1_c = np.stack([j_ids % w1c, j_ids // w1c], axis=-1).astype(np.int32)
    return conf, mask_v, j_ids, mkpts1_c, mconf
